# revision 7
# baseline (speedup 1.0000x reference)
"""Trainium2 Bass kernel for nn_GAT_30288109371812.

GAT(256->4x64) -> relu -> GAT(256->4x32) -> relu -> GCN(128->2) -> log_softmax
on N=50000 nodes, E=800000 edges (+self loops), 8 NeuronCores.

Strategy: edges sharded by destination node range (6250 dst nodes per
core). Host pre-sorts each core's edges into 128-node dst blocks
(degree-balanced block assignment), split into lo/hi src halves for the
int16 gather-index limit. On device, per dst block:
  - dma_gather fetches each edge's source-node table row (features +
    attention-src scores packed),
  - per-edge attention: adst expanded edge-wise with a one-hot-transpose
    matmul, z = asrc+adst, s = exp(leaky_relu(z)),
  - segment softmax numerator+denominator accumulate in PSUM via
    one-hot S matmuls (lhsT = S, rhs = [s*feat | s]),
  - fused epilogues: layer-2 projection (L2), dinv scaling (L3),
    W3 + log_softmax (L4).
Four SPMD launches; host gathers/reassembles node tables between layers.
"""

import numpy as np
import ml_dtypes
from contextlib import ExitStack

import concourse.bass as bass
import concourse.bacc as bacc
import concourse.mybir as mybir
from concourse import tile
from concourse.tile import ScopedClock
from concourse.bass_utils import run_bass_kernel_spmd

f32 = mybir.dt.float32
bf16 = mybir.dt.bfloat16
i16 = mybir.dt.int16
AL = mybir.AluOpType
ACTF = mybir.ActivationFunctionType
BF = ml_dtypes.bfloat16

# ---------------------------------------------------------------- patches
_MAX_WAITS = 1


def _drain_and_barrier_split(self, tick_clock, wait_clock):
    drain_inst = self.nc.sync.drain()
    wait_clock.add_sem_waits(
        drain_inst.ins, ScopedClock({None: tick_clock.global_clock})
    )
    si = drain_inst.ins.sync_info
    if si is not None and len(si.on_wait) > _MAX_WAITS:
        waits = list(si.on_wait)
        keep, rest = waits[:_MAX_WAITS], waits[_MAX_WAITS:]
        del si.on_wait[:]
        si.on_wait.extend(keep)
        while rest:
            chunk, rest = rest[:_MAX_WAITS], rest[_MAX_WAITS:]
            extra = self.nc.sync.drain()
            esi = extra.ins.sync_info
            if esi is None:
                extra.ins.sync_info = mybir.SyncInfo(
                    on_wait=list(chunk), on_update=[]
                )
            else:
                esi.on_wait.extend(chunk)
    self.nc.all_engine_barrier()
    assert self.sems is not None
    popped = self.nc._tile_sem_poison_stack.pop()
    assert popped is self._sem_poison
    self.nc.clear_and_free_semaphores(list(self.sems.allocated().values()))
    self.nc.all_engine_barrier()


tile.TileContext._drain_and_barrier = _drain_and_barrier_split


def _split_waits(nc, max_waits=_MAX_WAITS):
    """This walrus build rejects >1 sync wait per instruction; hoist the
    extras onto same-engine NoOps placed right before the instruction."""
    for f in nc.m.functions:
        for bb in f.blocks:
            new = []
            for ins in bb.instructions:
                si = ins.sync_info
                if si is not None and len(si.on_wait) > max_waits:
                    waits = list(si.on_wait)
                    extra, keep = waits[:-max_waits], waits[-max_waits:]
                    k = 0
                    while extra:
                        chunk, extra = extra[:max_waits], extra[max_waits:]
                        nop = mybir.InstNoOp(
                            name=f"{ins.name}-ws{k}", ins=[], outs=[]
                        )
                        nop.engine = ins.engine
                        nop.sync_info = mybir.SyncInfo(
                            on_wait=list(chunk), on_update=[]
                        )
                        new.append(nop)
                        k += 1
                    del si.on_wait[:]
                    si.on_wait.extend(keep)
                new.append(ins)
            bb.instructions[:] = new


# ---------------------------------------------------------------- config
N = 50000
F_IN = 256
H = 4
C1 = 64
C2 = 32
N_CLS = 2
E = 800000
NEG = 0.2
NCORES = 8
BLK = 128
NPC = N // NCORES            # 6250
NBLK = (NPC + BLK - 1) // BLK  # 49
NPC_PAD = NBLK * BLK         # 6272
LO_ROWS = 32768
HI_ROWS = N - LO_ROWS        # 17232

TW1, TW2, TW3 = 320, 192, 128  # table widths (f32,f32,bf16) rows %256B
C_L2, C_L3 = 256, 128
MW2, MW3 = 260, 132


def _wrap16(a):
    """int16 idx vector [L] -> [128, L//16]; idx i at [i%16, i//16],
    replicated across the 8 GPSIMD cores' 16-partition groups."""
    L = a.shape[0]
    w = a.reshape(L // 16, 16).T.astype(np.int16)
    return np.tile(w, (8, 1))


def _prep(edge_index):
    """Host-side edge preprocessing shared by launches 2/3/4."""
    src = np.asarray(edge_index[0], np.int64)
    dst = np.asarray(edge_index[1], np.int64)
    loops = np.arange(N, dtype=np.int64)
    src = np.concatenate([src, loops])
    dst = np.concatenate([dst, loops])
    deg = np.bincount(dst, minlength=N).astype(np.float32)

    cores = []
    # first pass: per-core block assignment (degree-balanced)
    for c in range(NCORES):
        m = (dst // NPC) == c
        es, ed = src[m], dst[m] - c * NPC
        islo = es < LO_ROWS
        cnt_lo = np.bincount(ed[islo], minlength=NPC)
        cnt_hi = np.bincount(ed[~islo], minlength=NPC)
        tot = cnt_lo + cnt_hi
        order = np.argsort(-tot, kind="stable")
        blk_lo = np.zeros(NBLK, np.int64)
        blk_hi = np.zeros(NBLK, np.int64)
        blk_n = np.zeros(NBLK, np.int64)
        slot_b = np.empty(NPC, np.int64)
        slot_r = np.empty(NPC, np.int64)
        for n_ in order:
            cand = np.where(blk_n < BLK)[0]
            b = cand[np.argmin((blk_lo + blk_hi)[cand])]
            slot_b[n_] = b
            slot_r[n_] = blk_n[b]
            blk_n[b] += 1
            blk_lo[b] += cnt_lo[n_]
            blk_hi[b] += cnt_hi[n_]
        cores.append(dict(es=es, ed=ed, islo=islo, slot_b=slot_b,
                          slot_r=slot_r, blk_lo=blk_lo, blk_hi=blk_hi))

    TLO = int(max((c["blk_lo"].max() for c in cores)) + BLK - 1) // BLK
    THI = int(max((c["blk_hi"].max() for c in cores)) + BLK - 1) // BLK
    TPB = TLO + THI
    G = (TPB + 3) // 4
    IW = NBLK * TPB * 8  # int16 cols of gather idx layout

    out = dict(TLO=TLO, THI=THI, TPB=TPB, G=G, IW=IW, deg=deg, cores=[])
    for c in range(NCORES):
        d = cores[c]
        es, ed, islo = d["es"], d["ed"], d["islo"]
        slot_b, slot_r = d["slot_b"], d["slot_r"]
        # perm[slot] = local node id, -1 for dummy slots
        perm = np.full(NPC_PAD, -1, np.int64)
        perm[slot_b * BLK + slot_r] = np.arange(NPC)
        # place edges
        gi = np.zeros((NBLK, TPB * BLK), np.int16)
        rel = np.full((NBLK, TPB * BLK), -1.0, np.float32)
        eb = slot_b[ed]
        er = slot_r[ed]
        for b in range(NBLK):
            for half, base_t, idx_off in ((True, 0, 0), (False, TLO, LO_ROWS)):
                sel = (eb == b) & (islo == half)
                k = int(sel.sum())
                if k == 0:
                    continue
                pos = base_t * BLK + np.arange(k)
                gi[b, pos] = (es[sel] - idx_off).astype(np.int16)
                rel[b, pos] = er[sel].astype(np.float32)
        # gather idx wrap layout per (block, half) window
        gidx = np.zeros((128, IW), np.int16)
        for b in range(NBLK):
            off = b * TPB * 8
            gidx[:, off:off + TLO * 8] = _wrap16(gi[b, :TLO * BLK])
            gidx[:, off + TLO * 8:off + TPB * 8] = _wrap16(gi[b, TLO * BLK:])
        # rel col layout [128, NBLK*TPB]: edge (b,t,p) -> [p, b*TPB+t]
        relc = rel.reshape(NBLK, TPB, BLK).transpose(2, 0, 1).reshape(128, NBLK * TPB)
        # relrow [NBLK, G*512]: row per block, rel in tile-major order
        relrow = np.full((NBLK, G * 512), -1.0, np.float32)
        relrow[:, :TPB * BLK] = rel
        # deg per slot
        degs = np.ones(NPC_PAD, np.float32)
        valid = perm >= 0
        degs[valid] = deg[c * NPC + perm[valid]]
        deg_sb = degs.reshape(NBLK, BLK).T.copy()  # [128, NBLK]
        out["cores"].append(dict(
            perm=perm, gidx=gidx,
            rel=relc.astype(BF), relrow=relrow.astype(BF),
            deg_sb=deg_sb,
        ))
    return out


# ---------------------------------------------------------------- builders
def _consts_pool(nc, tc, ctx, names):
    """Declare + load resident const inputs; returns dict name->sbuf tile."""
    pool = ctx.enter_context(tc.tile_pool(name="consts", bufs=1))
    tiles = {}
    for name, shape, dt in names:
        d = nc.declare_dram_parameter(name, list(shape), dt, isOutput=False)
        t = pool.tile(list(shape), dt, tag=name)
        nc.sync.dma_start(out=t[:], in_=d[:])
        tiles[name] = t
    return tiles


def build_l1(TPB_unused=None):
    nc = bacc.Bacc("TRN2", target_bir_lowering=False, debug=False,
                   num_devices=NCORES)
    xT = nc.declare_dram_parameter("xT", [F_IN, NPC_PAD], f32, isOutput=False)
    w1 = nc.declare_dram_parameter("w1ext", [F_IN, MW2 + 4], f32, isOutput=False)
    out_d = nc.declare_dram_parameter("h1sc", [NPC_PAD, MW2 + 4], f32, isOutput=True)
    OW = MW2 + 4  # 264
    with tile.TileContext(nc) as tc, ExitStack() as ctx:
        cpool = ctx.enter_context(tc.tile_pool(name="c", bufs=1))
        pool = ctx.enter_context(tc.tile_pool(name="w", bufs=3))
        psum = ctx.enter_context(tc.tile_pool(name="p", bufs=4, space="PSUM"))
        sb_x = cpool.tile([128, 2, NPC_PAD], f32)
        nc.sync.dma_start(out=sb_x[:, 0, :], in_=xT[0:128, :])
        nc.sync.dma_start(out=sb_x[:, 1, :], in_=xT[128:256, :])
        sb_w = cpool.tile([128, 2, OW], f32)
        nc.sync.dma_start(out=sb_w[:, 0, :], in_=w1[0:128, :])
        nc.sync.dma_start(out=sb_w[:, 1, :], in_=w1[128:256, :])
        for nb in range(NBLK):
            ps = psum.tile([128, OW], f32)
            for k in range(2):
                nc.tensor.matmul(ps[:], sb_x[:, k, nb * 128:(nb + 1) * 128],
                                 sb_w[:, k, :], start=(k == 0), stop=(k == 1))
            o = pool.tile([128, OW], f32)
            nc.vector.tensor_copy(o[:], ps[:])
            nc.sync.dma_start(out=out_d[nb * 128:(nb + 1) * 128, :], in_=o[:])
    nc.finalize()
    _split_waits(nc)
    return nc


def build_gat(layer, TLO, THI, G):
    """layer 2: C=256,H=4,CH=64,TW=320, fused proj -> out [NPC_PAD,136]
       layer 3: C=128,H=4,CH=32,TW=192, dinv scale -> out [NPC_PAD,128]"""
    TPB = TLO + THI
    if layer == 2:
        C, CH, TW, MW = C_L2, C1, TW1, MW2
        OUTW = 136
    else:
        C, CH, TW, MW = C_L3, C2, TW2, MW3
        OUTW = 128
    IW = NBLK * TPB * 8
    nc = bacc.Bacc("TRN2", target_bir_lowering=False, debug=False,
                   num_devices=NCORES)
    tlo_d = nc.declare_dram_parameter("tlo", [LO_ROWS, TW], f32, isOutput=False)
    thi_d = nc.declare_dram_parameter("thi", [HI_ROWS, TW], f32, isOutput=False)
    relrow_d = nc.declare_dram_parameter("relrow", [NBLK, G * 512], bf16,
                                         isOutput=False)
    out_d = nc.declare_dram_parameter("out", [NPC_PAD, OUTW], f32, isOutput=True)

    with tile.TileContext(nc) as tc, ExitStack() as ctx:
        names = [
            ("gidx", (128, IW), i16),
            ("rel", (128, NBLK * TPB), bf16),

            ("adst", (128, NBLK * H), f32),
            ("bias", (128, C), f32),
            ("iota_col", (128, 1), f32),
            ("iota_row", (128, 128), bf16),
            ("ones1", (1, 128), bf16),
        ]
        if layer == 2:
            names += [("w2ext", (128, 2, OUTW), f32), ("ident", (128, 128), f32)]
        else:
            names += [("deg", (128, NBLK), f32)]
        cs = _consts_pool(nc, tc, ctx, names)

        pool = ctx.enter_context(tc.tile_pool(name="w", bufs=2))
        spool = ctx.enter_context(tc.tile_pool(name="s", bufs=3))
        ps_acc_p = ctx.enter_context(tc.tile_pool(name="pacc", bufs=2, space="PSUM"))
        ps_ad_p = ctx.enter_context(tc.tile_pool(name="pad", bufs=2, space="PSUM"))
        ps_bc_p = ctx.enter_context(tc.tile_pool(name="pbc", bufs=1, space="PSUM"))
        ps_tr_p = ctx.enter_context(tc.tile_pool(name="ptr", bufs=1, space="PSUM"))
        ps_h2_p = ctx.enter_context(tc.tile_pool(name="ph2", bufs=1, space="PSUM"))

        GCH = 8  # max tiles (1024 idxs) per dma_gather: larger crashes SWDGE
        for nb in range(NBLK):
            ioff = nb * TPB * 8
            glo = pool.tile([128, TLO, TW], f32)
            for q0 in range(0, TLO, GCH):
                qn = min(GCH, TLO - q0)
                nc.gpsimd.dma_gather(glo[:, q0:q0 + qn, :], tlo_d[:, :],
                                     cs["gidx"][:, ioff + q0 * 8:ioff + (q0 + qn) * 8],
                                     qn * 128, qn * 128, TW)
            ghi = pool.tile([128, THI, TW], f32)
            for q0 in range(0, THI, GCH):
                qn = min(GCH, THI - q0)
                nc.gpsimd.dma_gather(ghi[:, q0:q0 + qn, :], thi_d[:, :],
                                     cs["gidx"][:, ioff + (TLO + q0) * 8:ioff + (TLO + q0 + qn) * 8],
                                     qn * 128, qn * 128, TW)

            # one-hot S for all tiles of the block
            S_all = pool.tile([128, TPB, 128], bf16)
            iota_rep = cs["iota_row"][:, :].unsqueeze(1).broadcast_to([128, TPB, 128])
            rel_rep = cs["rel"][:, nb * TPB:(nb + 1) * TPB].unsqueeze(2) \
                .broadcast_to([128, TPB, 128])
            nc.vector.tensor_tensor(S_all[:], iota_rep, rel_rep, op=AL.is_equal)

            # adst edge-expansion: partition-bcast rel rows + eq -> S^T,
            # then S^T @ adst_block
            rr = spool.tile([1, G * 512], bf16)
            nc.sync.dma_start(out=rr[:], in_=relrow_d[nb:nb + 1, :])
            ps_ad = ps_ad_p.tile([128, TPB * H], f32)
            for g in range(G):
                ps_bc = ps_bc_p.tile([128, 512], f32)
                nc.tensor.matmul(ps_bc[:], cs["ones1"][:],
                                 rr[:, g * 512:(g + 1) * 512],
                                 start=True, stop=True)
                ST = spool.tile([128, 512], f32)
                nc.vector.tensor_scalar(ST[:], ps_bc[:], cs["iota_col"][:, :],
                                        None, op0=AL.is_equal)
                for j in range(min(4, TPB - g * 4)):
                    t = g * 4 + j
                    nc.tensor.matmul(ps_ad[:, t * H:(t + 1) * H],
                                     ST[:, j * 128:(j + 1) * 128],
                                     cs["adst"][:, nb * H:(nb + 1) * H],
                                     start=True, stop=True)

            # z = asrc + adst ; s = exp(leaky(z)) written into Mp s-cols
            z = pool.tile([128, TPB, H], f32)
            nc.vector.tensor_tensor(
                z[:, 0:TLO, :],
                ps_ad[:, 0:TLO * H].rearrange("p (t h) -> p t h", h=H),
                glo[:, :, C:C + H], op=AL.add)
            nc.vector.tensor_tensor(
                z[:, TLO:TPB, :],
                ps_ad[:, TLO * H:TPB * H].rearrange("p (t h) -> p t h", h=H),
                ghi[:, :, C:C + H], op=AL.add)
            nc.vector.scalar_tensor_tensor(z[:], z[:], NEG, z[:],
                                           op0=AL.mult, op1=AL.max)
            Mp = pool.tile([128, TPB, MW], bf16)
            nc.scalar.activation(Mp[:, :, C:C + H], z[:], ACTF.Exp)

            # scale features by s
            s_lo = Mp[:, 0:TLO, C:C + H].unsqueeze(3).broadcast_to([128, TLO, H, CH])
            nc.vector.tensor_tensor(
                Mp[:, 0:TLO, 0:C].rearrange("p t (h c) -> p t h c", h=H),
                glo[:, 0:TLO, 0:C].rearrange("p t (h c) -> p t h c", h=H),
                s_lo, op=AL.mult)
            s_hi = Mp[:, TLO:TPB, C:C + H].unsqueeze(3).broadcast_to([128, THI, H, CH])
            nc.vector.tensor_tensor(
                Mp[:, TLO:TPB, 0:C].rearrange("p t (h c) -> p t h c", h=H),
                ghi[:, 0:THI, 0:C].rearrange("p t (h c) -> p t h c", h=H),
                s_hi, op=AL.mult)

            # segment numerator+denominator
            ps_acc = ps_acc_p.tile([128, MW], f32)
            for t in range(TPB):
                nc.tensor.matmul(ps_acc[:], S_all[:, t, :], Mp[:, t, :],
                                 start=(t == 0), stop=(t == TPB - 1))

            # epilogue
            rcp = spool.tile([128, H], f32)
            nc.vector.reciprocal(rcp[:], ps_acc[:, C:C + H])
            relu = pool.tile([128, C], f32)
            r_rep = rcp[:, :].unsqueeze(2).broadcast_to([128, H, CH])
            nc.vector.tensor_tensor(
                relu[:].rearrange("p (h c) -> p h c", h=H),
                ps_acc[:, 0:C].rearrange("p (h c) -> p h c", h=H),
                r_rep, op=AL.mult)
            nc.vector.tensor_tensor(relu[:], relu[:], cs["bias"][:], op=AL.add)
            nc.vector.tensor_scalar(relu[:], relu[:], 0.0, None, op0=AL.max)

            outt = spool.tile([128, OUTW], f32)
            if layer == 2:
                ps_h2 = ps_h2_p.tile([128, OUTW], f32)
                trT = spool.tile([128, 2, 128], f32)
                for k in range(2):
                    ps_tr = ps_tr_p.tile([128, 128], f32)
                    nc.tensor.transpose(ps_tr[:], relu[:, k * 128:(k + 1) * 128],
                                        cs["ident"][:])
                    nc.vector.tensor_copy(trT[:, k, :], ps_tr[:])
                for k in range(2):
                    nc.tensor.matmul(ps_h2[:], trT[:, k, :], cs["w2ext"][:, k, :],
                                     start=(k == 0), stop=(k == 1))
                nc.vector.tensor_copy(outt[:], ps_h2[:])
            else:
                sq = spool.tile([128, 1], f32)
                nc.scalar.activation(sq[:], cs["deg"][:, nb:nb + 1], ACTF.Sqrt)
                dv = spool.tile([128, 1], f32)
                nc.vector.reciprocal(dv[:], sq[:])
                nc.vector.tensor_scalar(outt[:], relu[:], dv[:, :], None,
                                        op0=AL.mult)
            nc.sync.dma_start(out=out_d[nb * 128:(nb + 1) * 128, :], in_=outt[:])
    nc.finalize()
    _split_waits(nc)
    return nc


def build_l4(TLO, THI, G):
    TPB = TLO + THI
    TW = TW3
    IW = NBLK * TPB * 8
    nc = bacc.Bacc("TRN2", target_bir_lowering=False, debug=False,
                   num_devices=NCORES)
    tlo_d = nc.declare_dram_parameter("tlo", [LO_ROWS, TW], bf16, isOutput=False)
    thi_d = nc.declare_dram_parameter("thi", [HI_ROWS, TW], bf16, isOutput=False)
    out_d = nc.declare_dram_parameter("out", [NPC_PAD, N_CLS], f32, isOutput=True)
    with tile.TileContext(nc) as tc, ExitStack() as ctx:
        names = [
            ("gidx", (128, IW), i16),
            ("rel", (128, NBLK * TPB), bf16),
            ("iota_row", (128, 128), bf16),
            ("ident_bf", (128, 128), bf16),
            ("deg", (128, NBLK), f32),
            ("w3", (128, N_CLS), bf16),
            ("b3", (128, N_CLS), f32),
        ]
        cs = _consts_pool(nc, tc, ctx, names)
        pool = ctx.enter_context(tc.tile_pool(name="w", bufs=2))
        spool = ctx.enter_context(tc.tile_pool(name="s", bufs=3))
        psum2 = ctx.enter_context(tc.tile_pool(name="p2", bufs=2, space="PSUM"))
        GCH = 8
        for nb in range(NBLK):
            ioff = nb * TPB * 8
            glo = pool.tile([128, TLO, TW], bf16)
            for q0 in range(0, TLO, GCH):
                qn = min(GCH, TLO - q0)
                nc.gpsimd.dma_gather(glo[:, q0:q0 + qn, :], tlo_d[:, :],
                                     cs["gidx"][:, ioff + q0 * 8:ioff + (q0 + qn) * 8],
                                     qn * 128, qn * 128, TW)
            ghi = pool.tile([128, THI, TW], bf16)
            for q0 in range(0, THI, GCH):
                qn = min(GCH, THI - q0)
                nc.gpsimd.dma_gather(ghi[:, q0:q0 + qn, :], thi_d[:, :],
                                     cs["gidx"][:, ioff + (TLO + q0) * 8:ioff + (TLO + q0 + qn) * 8],
                                     qn * 128, qn * 128, TW)
            S_all = pool.tile([128, TPB, 128], bf16)
            iota_rep = cs["iota_row"][:, :].unsqueeze(1).broadcast_to([128, TPB, 128])
            rel_rep = cs["rel"][:, nb * TPB:(nb + 1) * TPB].unsqueeze(2) \
                .broadcast_to([128, TPB, 128])
            nc.vector.tensor_tensor(S_all[:], iota_rep, rel_rep, op=AL.is_equal)
            ps_acc = psum2.tile([128, TW], f32)
            for t in range(TPB):
                src = glo[:, t, :] if t < TLO else ghi[:, t - TLO, :]
                nc.tensor.matmul(ps_acc[:], S_all[:, t, :], src,
                                 start=(t == 0), stop=(t == TPB - 1))
            # dinv_d scale -> transpose -> @W3 -> +b3 -> log_softmax
            sq = spool.tile([128, 1], f32)
            nc.scalar.activation(sq[:], cs["deg"][:, nb:nb + 1], ACTF.Sqrt)
            dv = spool.tile([128, 1], f32)
            nc.vector.reciprocal(dv[:], sq[:])
            aggd = pool.tile([128, TW], bf16)
            nc.vector.tensor_scalar(aggd[:], ps_acc[:], dv[:, :], None, op0=AL.mult)
            ps_tr = psum2.tile([128, 128], bf16)
            nc.tensor.transpose(ps_tr[:], aggd[:], cs["ident_bf"][:])
            trT = spool.tile([128, 128], bf16)
            nc.vector.tensor_copy(trT[:], ps_tr[:])
            ps_o = psum2.tile([128, N_CLS], f32)
            nc.tensor.matmul(ps_o[:], trT[:], cs["w3"][:], start=True, stop=True)
            o = spool.tile([128, N_CLS], f32)
            nc.vector.tensor_tensor(o[:], ps_o[:], cs["b3"][:], op=AL.add)
            mx = spool.tile([128, 1], f32)
            nc.vector.tensor_reduce(mx[:], o[:], axis=mybir.AxisListType.X, op=AL.max)
            tshift = spool.tile([128, N_CLS], f32)
            nc.vector.tensor_scalar(tshift[:], o[:], mx[:, :], None, op0=AL.subtract)
            ex = spool.tile([128, N_CLS], f32)
            nc.scalar.activation(ex[:], tshift[:], ACTF.Exp)
            se = spool.tile([128, 1], f32)
            nc.vector.tensor_reduce(se[:], ex[:], axis=mybir.AxisListType.X, op=AL.add)
            ln = spool.tile([128, 1], f32)
            nc.scalar.activation(ln[:], se[:], ACTF.Ln)
            res = spool.tile([128, N_CLS], f32)
            nc.vector.tensor_scalar(res[:], tshift[:], ln[:, :], None,
                                    op0=AL.subtract)
            nc.sync.dma_start(out=out_d[nb * 128:(nb + 1) * 128, :], in_=res[:])
    nc.finalize()
    _split_waits(nc)
    return nc


# ---------------------------------------------------------------- host glue
_NC_CACHE = {}


def _get_nc(key, builder, *args):
    if key not in _NC_CACHE:
        _NC_CACHE[key] = builder(*args)
    return _NC_CACHE[key]


def _att_pack(att_src, att_dst, C, CH):
    """Block-diagonal [C, 2H] so h @ A gives [asrc | adst] per head."""
    A = np.zeros((C, 2 * H), np.float32)
    for h in range(H):
        A[h * CH:(h + 1) * CH, h] = att_src[h]
        A[h * CH:(h + 1) * CH, H + h] = att_dst[h]
    return A


def _run(nc, in_maps, trace=False):
    return run_bass_kernel_spmd(nc, in_maps, list(range(NCORES)), trace=trace)


_LAST_EXEC_NS = {}


def kernel(x, edge_index, W1, att_src1, att_dst1, b1, W2, att_src2, att_dst2,
           b2, W3, b3, _trace=False):
    x = np.asarray(x, np.float32)
    prep = _prep(np.asarray(edge_index))
    TLO, THI, G = prep["TLO"], prep["THI"], prep["G"]
    TPB = TLO + THI

    iota_col = np.arange(128, dtype=np.float32).reshape(128, 1)
    iota_row = np.tile(np.arange(128, dtype=np.float32), (128, 1)).astype(BF)
    ones1 = np.ones((1, 128), BF)
    ident = np.eye(128, dtype=np.float32)
    ident_bf = np.eye(128, dtype=np.float32).astype(BF)

    # ---------------- launch 1: h1 = x@W1 (+ attention scores)
    A1 = _att_pack(np.asarray(att_src1, np.float32),
                   np.asarray(att_dst1, np.float32), C_L2, C1)
    W1f = np.asarray(W1, np.float32)
    w1ext = np.concatenate([W1f, W1f @ A1], axis=1)  # [256,264]
    xT = np.ascontiguousarray(x.T)  # [256, N]
    in_maps = []
    for c in range(NCORES):
        sl = np.zeros((F_IN, NPC_PAD), np.float32)
        sl[:, :NPC] = xT[:, c * NPC:(c + 1) * NPC]
        in_maps.append({"xT": sl, "w1ext": w1ext})
    nc1 = _get_nc("l1", build_l1)
    r1 = _run(nc1, in_maps, trace=_trace)
    _LAST_EXEC_NS["l1"] = r1.exec_time_ns

    h1 = np.concatenate([r1.results[c]["h1sc"][:NPC] for c in range(NCORES)], 0)
    # ext1 table [N, 320] = [h1 | asrc1 | pad]
    ext1 = np.zeros((N, TW1), np.float32)
    ext1[:, :C_L2] = h1[:, :C_L2]
    ext1[:, C_L2:C_L2 + H] = h1[:, MW2 - 4 + 0:MW2 + 0]  # cols 256..260 asrc
    adst1_full = h1[:, MW2:MW2 + 4]                      # cols 260..264

    # ---------------- launch 2: GAT1 + fused W2 projection
    A2 = _att_pack(np.asarray(att_src2, np.float32),
                   np.asarray(att_dst2, np.float32), C_L3, C2)
    W2f = np.asarray(W2, np.float32)
    w2ext = np.concatenate([W2f, W2f @ A2], 1)  # [256,136]
    w2ext_k = np.ascontiguousarray(
        w2ext.reshape(2, 128, 136).transpose(1, 0, 2))  # [128,2,136]
    b1bc = np.tile(np.asarray(b1, np.float32), (128, 1))
    nc2 = _get_nc(("gat", 2, TLO, THI), build_gat, 2, TLO, THI, G)
    in_maps = []
    for c in range(NCORES):
        pc = prep["cores"][c]
        perm = pc["perm"]
        adst_sb = np.zeros((128, NBLK * H), np.float32)
        valid = perm >= 0
        av = np.zeros((NPC_PAD, H), np.float32)
        av[valid] = adst1_full[c * NPC + perm[valid]]
        adst_sb[:, :] = av.reshape(NBLK, BLK, H).transpose(1, 0, 2).reshape(128, NBLK * H)
        in_maps.append({
            "tlo": ext1[:LO_ROWS], "thi": ext1[LO_ROWS:],
            "gidx": pc["gidx"], "rel": pc["rel"], "relrow": pc["relrow"],
            "adst": adst_sb, "bias": b1bc, "iota_col": iota_col,
            "iota_row": iota_row, "ones1": ones1,
            "w2ext": w2ext_k, "ident": ident,
        })
    r2 = _run(nc2, in_maps, trace=_trace)
    _LAST_EXEC_NS["l2"] = r2.exec_time_ns

    # reassemble h2/asrc2/adst2
    ext2 = np.zeros((N, TW2), np.float32)
    adst2_full = np.zeros((N, H), np.float32)
    for c in range(NCORES):
        o = r2.results[c]["out"]
        perm = prep["cores"][c]["perm"]
        valid = perm >= 0
        gl = c * NPC + perm[valid]
        ext2[gl, :C_L3] = o[valid, :C_L3]
        ext2[gl, C_L3:C_L3 + H] = o[valid, C_L3:C_L3 + H]
        adst2_full[gl] = o[valid, C_L3 + H:C_L3 + 2 * H]

    # ---------------- launch 3: GAT2 + dinv scaling
    b2bc = np.tile(np.asarray(b2, np.float32), (128, 1))
    nc3 = _get_nc(("gat", 3, TLO, THI), build_gat, 3, TLO, THI, G)
    in_maps = []
    for c in range(NCORES):
        pc = prep["cores"][c]
        perm = pc["perm"]
        valid = perm >= 0
        av = np.zeros((NPC_PAD, H), np.float32)
        av[valid] = adst2_full[c * NPC + perm[valid]]
        adst_sb = av.reshape(NBLK, BLK, H).transpose(1, 0, 2).reshape(128, NBLK * H)
        in_maps.append({
            "tlo": ext2[:LO_ROWS], "thi": ext2[LO_ROWS:],
            "gidx": pc["gidx"], "rel": pc["rel"], "relrow": pc["relrow"],
            "adst": adst_sb, "bias": b2bc, "iota_col": iota_col,
            "iota_row": iota_row, "ones1": ones1, "deg": pc["deg_sb"],
        })
    r3 = _run(nc3, in_maps, trace=_trace)
    _LAST_EXEC_NS["l3"] = r3.exec_time_ns

    tab3 = np.zeros((N, TW3), np.float32)
    for c in range(NCORES):
        o = r3.results[c]["out"]
        perm = prep["cores"][c]["perm"]
        valid = perm >= 0
        tab3[c * NPC + perm[valid]] = o[valid]
    tab3 = tab3.astype(BF)

    # ---------------- launch 4: GCN + log_softmax
    w3b = np.zeros((128, N_CLS), np.float32)
    w3b[:C_L3] = np.asarray(W3, np.float32)
    w3b = w3b.astype(BF)
    b3bc = np.tile(np.asarray(b3, np.float32), (128, 1))
    nc4 = _get_nc(("l4", TLO, THI), build_l4, TLO, THI, G)
    in_maps = []
    for c in range(NCORES):
        pc = prep["cores"][c]
        in_maps.append({
            "tlo": tab3[:LO_ROWS], "thi": tab3[LO_ROWS:],
            "gidx": pc["gidx"], "rel": pc["rel"], "iota_row": iota_row,
            "ident_bf": ident_bf, "deg": pc["deg_sb"], "w3": w3b, "b3": b3bc,
        })
    r4 = _run(nc4, in_maps, trace=_trace)
    _LAST_EXEC_NS["l4"] = r4.exec_time_ns

    out = np.zeros((N, N_CLS), np.float32)
    for c in range(NCORES):
        o = r4.results[c]["out"]
        perm = prep["cores"][c]["perm"]
        valid = perm >= 0
        out[c * NPC + perm[valid]] = o[valid]
    return out
